# revision 2
# baseline (speedup 1.0000x reference)
"""KMeans assignment kernel for TRN2 (8 NeuronCores, data-parallel over points).

Computes argmin_k ||x_n - c_k||^2 for x (65536, 512) f32, centers (4096, 512) f32.

Strategy:
  - argmin_k dist = argmax_k s,  s = 2*x.c_k - ||c_k||^2   (x-norm constant per row)
  - matmul p = (-2x) @ c^T on the PE in fp32r (e8m11) with a hi/lo split:
        v = -2x;  v = v_hi + v_lo;  c = c_hi + c_lo   (each part exactly e8m11)
        p = v_hi.c_hi + v_hi.c_lo + v_lo.c_hi         (drops v_lo.c_lo ~ 1e-8 rel)
    giving fp32-level dot-product accuracy at 3 passes x full PE rate
    (native fp32 matmul is 4x slower per pass).
  - DVE: s = (-c_norm) - p per PSUM bank, then max + max_index over K=4096
    (first-match tie-break == jnp.argmin first-min tie-break after negation).
  - Data-parallel: shard points across 8 cores (8192 points/core), centers
    replicated; no collectives needed.
"""
import os
import numpy as np

import concourse.bass as bass
import concourse.bacc as bacc
import concourse.tile as tile
import concourse.mybir as mybir
from concourse.bass_utils import run_bass_kernel_spmd

N_CORES = 8
N_POINTS = 65536
K = 4096
F = 512
PTS_PER_CORE = N_POINTS // N_CORES      # 8192
NT = PTS_PER_CORE // 128                # 64 x-tiles per core
NFC = F // 128                          # 4 contraction chunks
NB = K // 512                           # 8 PSUM banks
F32 = mybir.dt.float32
F32R = mybir.dt.float32r
U32 = mybir.dt.uint32

_NC = None
LAST_BR = None  # BassKernelResults of the last run (for test harness timing)


def round_fp32r(a: np.ndarray) -> np.ndarray:
    """Round f32 to fp32r (e8m11): RNE to 11 mantissa bits; low 12 bits zero.

    Matches the PE's interpretation of fp32r operands bit-exactly (verified
    on hardware against walrus's cast_fp32_to_fp32r)."""
    bits = np.ascontiguousarray(a, dtype=np.float32).view(np.uint32)
    rounded = (bits.astype(np.uint64) + 0x7FF + ((bits >> 12) & 1)) & 0xFFFFF000
    return rounded.astype(np.uint32).view(np.float32)


def _build():
    nc = bacc.Bacc("TRN2", target_bir_lowering=False, debug=False,
                   num_devices=N_CORES)
    xh_d = nc.declare_dram_parameter("xh", [NT, 128, NFC, 128], F32R, isOutput=False)
    xl_d = nc.declare_dram_parameter("xl", [NT, 128, NFC, 128], F32R, isOutput=False)
    ch_d = nc.declare_dram_parameter("ch", [128, NFC, K], F32R, isOutput=False)
    cl_d = nc.declare_dram_parameter("cl", [128, NFC, K], F32R, isOutput=False)
    cnn_d = nc.declare_dram_parameter("cnn", [128, K], F32, isOutput=False)
    out_d = nc.declare_dram_parameter("oidx", [128, NT], U32, isOutput=True)

    with tile.TileContext(nc) as tc:
        with (
            tc.tile_pool(name="const", bufs=1) as cpool,
            tc.tile_pool(name="xp", bufs=3) as xpool,
            tc.tile_pool(name="sp", bufs=2) as spool,
            tc.tile_pool(name="mp", bufs=2) as mpool,
            tc.tile_pool(name="st", bufs=1) as stpool,
            tc.tile_pool(name="ps", bufs=1, space="PSUM") as pspool,
        ):
            ch = cpool.tile([128, NFC, K], F32R, tag="ch")
            cl = cpool.tile([128, NFC, K], F32R, tag="cl")
            cnn = cpool.tile([128, K], F32, tag="cnn")
            nc.sync.dma_start(ch[:], ch_d[:])
            nc.sync.dma_start(cl[:], cl_d[:])
            nc.sync.dma_start(cnn[:], cnn_d[:])

            stg8 = stpool.tile([128, NT, 8], U32, tag="stg8")

            for t in range(NT):
                xh = xpool.tile([128, NFC * 128], F32R, tag="xh")
                xl = xpool.tile([128, NFC * 128], F32R, tag="xl")
                nc.sync.dma_start(xh[:], xh_d[t])
                nc.sync.dma_start(xl[:], xl_d[t])

                p = pspool.tile([128, K], F32, tag="p")
                s = spool.tile([128, K], F32, tag="s")
                for kc in range(NB):
                    ks = slice(kc * 512, (kc + 1) * 512)
                    mms = (
                        [(xh, ch, fc) for fc in range(NFC)]
                        + [(xh, cl, fc) for fc in range(NFC)]
                        + [(xl, ch, fc) for fc in range(NFC)]
                    )
                    for i, (w, c, fc) in enumerate(mms):
                        nc.tensor.matmul(
                            p[:, ks],
                            w[:, fc * 128:(fc + 1) * 128],
                            c[:, fc, ks],
                            start=(i == 0),
                            stop=(i == len(mms) - 1),
                        )
                    # s = (-c_norm) - p  == -(dist - x_norm); max s == min dist
                    nc.vector.tensor_tensor(
                        out=s[:, ks], in0=cnn[:, ks], in1=p[:, ks],
                        op=mybir.AluOpType.subtract,
                    )
                m8 = mpool.tile([128, 8], F32, tag="m8")
                nc.vector.max(m8[:], s[:])
                nc.vector.max_index(stg8[:, t, :], m8[:], s[:])

            ex = stpool.tile([128, NT], U32, tag="ex")
            nc.vector.tensor_copy(ex[:], stg8[:, :, 0])
            nc.gpsimd.dma_start(out_d[:], ex[:])
    nc.compile()
    return nc


def _get_nc():
    global _NC
    if _NC is None:
        _NC = _build()
    return _NC


def kernel(x: np.ndarray, centers: np.ndarray) -> np.ndarray:
    global LAST_BR
    x = np.ascontiguousarray(x, dtype=np.float32)
    centers = np.ascontiguousarray(centers, dtype=np.float32)

    # hi/lo fp32r split of v = -2x and c
    v = (-2.0 * x).astype(np.float32)
    v_hi = round_fp32r(v)
    v_lo = round_fp32r((v - v_hi).astype(np.float32))
    c_hi = round_fp32r(centers)
    c_lo = round_fp32r((centers - c_hi).astype(np.float32))

    # pack x side: [core, t, fp, fc, j] <- v[core*8192 + t*128 + j, fc*128 + fp]
    def pack_x(a):
        a = a.reshape(N_CORES, NT, 128, NFC, 128)        # [core, t, j, fc, fp]
        return np.ascontiguousarray(a.transpose(0, 1, 4, 3, 2))

    xh_p = pack_x(v_hi)
    xl_p = pack_x(v_lo)

    # pack c side: [fp, fc, k] <- c[k, fc*128 + fp]
    def pack_c(a):
        a = a.reshape(K, NFC, 128)                        # [k, fc, fp]
        return np.ascontiguousarray(a.transpose(2, 1, 0))

    ch_p = pack_c(c_hi)
    cl_p = pack_c(c_lo)

    c_norm = (centers.astype(np.float64) ** 2).sum(axis=1).astype(np.float32)
    cnn_p = np.ascontiguousarray(
        np.broadcast_to(-c_norm[None, :], (128, K)).astype(np.float32))

    in_maps = [
        {"xh": xh_p[i], "xl": xl_p[i], "ch": ch_p, "cl": cl_p, "cnn": cnn_p}
        for i in range(N_CORES)
    ]

    nc = _get_nc()
    global _LAST_IN_MAPS
    _LAST_IN_MAPS = in_maps
    br = run_bass_kernel_spmd(nc, in_maps, list(range(N_CORES)))
    LAST_BR = br

    parts = []
    for i in range(N_CORES):
        oidx = br.results[i]["oidx"]                      # (128, NT) u32
        parts.append(oidx.T.reshape(-1))                  # point-major
    return np.concatenate(parts).astype(np.int32)


_LAST_IN_MAPS = None


def _make_runner(nc, in_maps):
    """Build a reusable sharded-jit runner with device-resident inputs.

    Mirrors bass2jax.run_bass_via_pjrt's multi-core path, but keeps the big
    inputs on device so repeated calls measure execution, not transfer."""
    import jax
    from jax.sharding import Mesh, PartitionSpec
    from jax.experimental.shard_map import shard_map
    from concourse import bass2jax
    from concourse.bass2jax import _bass_exec_p, partition_id_tensor

    bass2jax.install_neuronx_cc_hook()
    n_cores = len(in_maps)
    partition_name = nc.partition_id_tensor.name if nc.partition_id_tensor else None
    in_names, out_names, out_avals, zero_outs = [], [], [], []
    for alloc in nc.m.functions[0].allocations:
        if not isinstance(alloc, mybir.MemoryLocationSet):
            continue
        name = alloc.memorylocations[0].name
        if alloc.kind == "ExternalInput":
            if name != partition_name:
                in_names.append(name)
        elif alloc.kind == "ExternalOutput":
            shape = tuple(alloc.tensor_shape)
            dtype = mybir.dt.np(alloc.dtype)
            out_names.append(name)
            out_avals.append(jax.core.ShapedArray(shape, dtype))
            zero_outs.append(np.zeros(shape, dtype))
    n_params = len(in_names)
    all_in_names = list(in_names) + list(out_names)
    if partition_name is not None:
        all_in_names.append(partition_name)
    donate = tuple(range(n_params, n_params + len(out_names)))

    def _body(*args):
        operands = list(args)
        if partition_name is not None:
            operands.append(partition_id_tensor())
        return tuple(_bass_exec_p.bind(
            *operands,
            out_avals=tuple(out_avals),
            in_names=tuple(all_in_names),
            out_names=tuple(out_names),
            lowering_input_output_aliases=(),
            sim_require_finite=True,
            sim_require_nnan=True,
            nc=nc,
        ))

    devices = jax.devices()[:n_cores]
    mesh = Mesh(np.asarray(devices), ("core",))
    in_specs = (PartitionSpec("core"),) * (n_params + len(out_names))
    out_specs = (PartitionSpec("core"),) * len(out_names)
    sharded = jax.jit(
        shard_map(_body, mesh=mesh, in_specs=in_specs, out_specs=out_specs,
                  check_rep=False),
        donate_argnums=donate, keep_unused=True)

    from jax.sharding import NamedSharding
    concat_in = []
    for i, name in enumerate(in_names):
        arr = np.concatenate([np.asarray(m[name]) for m in in_maps], axis=0)
        sh = NamedSharding(mesh, PartitionSpec("core"))
        concat_in.append(jax.device_put(arr, sh))

    def run():
        import jax
        czeros = [np.zeros((n_cores * z.shape[0], *z.shape[1:]), z.dtype)
                  for z in zero_outs]
        outs = sharded(*concat_in, *czeros)
        jax.block_until_ready(outs)
        return outs

    return run


def measure_exec_ns(reps: int = 12) -> int:
    """Estimate per-call HW execution time: steady-state wall minus the
    dispatch overhead of a null kernel."""
    import time
    nc = _get_nc()
    in_maps = _LAST_IN_MAPS
    assert in_maps is not None, "call kernel() first"
    run = _make_runner(nc, in_maps)
    run()  # warm
    times = []
    for _ in range(reps):
        t0 = time.perf_counter()
        run()
        times.append(time.perf_counter() - t0)
    t_kernel = min(times)

    # null kernel for dispatch overhead
    nc0 = bacc.Bacc("TRN2", target_bir_lowering=False, debug=False,
                    num_devices=N_CORES)
    a_d = nc0.declare_dram_parameter("a", [128, 1], F32, isOutput=False)
    b_d = nc0.declare_dram_parameter("b", [128, 1], F32, isOutput=True)
    with tile.TileContext(nc0) as tc:
        with tc.tile_pool(name="p", bufs=1) as pool:
            tt = pool.tile([128, 1], F32, tag="t")
            nc0.sync.dma_start(tt[:], a_d[:])
            nc0.gpsimd.dma_start(b_d[:], tt[:])
    nc0.compile()
    run0 = _make_runner(nc0, [{"a": np.zeros((128, 1), np.float32)}] * N_CORES)
    run0()
    times0 = []
    for _ in range(reps):
        t0 = time.perf_counter()
        run0()
        times0.append(time.perf_counter() - t0)
    t_null = min(times0)
    print(f"  [timing] steady-state wall: {t_kernel*1e6:.1f}us, "
          f"null-kernel wall: {t_null*1e6:.1f}us")
    return int((t_kernel - t_null) * 1e9)
